# revision 1
# baseline (speedup 1.0000x reference)
"""Causal self-attention (RoPE + QK-RMSNorm, GQA 16q/8kv) Trainium2 Bass kernel.

Sharding: 8 cores = 2 batch x 4 tensor-parallel. Core c handles batch b=c//4 and
q-heads [4*tp, 4*tp+4), kv-heads [2*tp, 2*tp+2) where tp=c%4. Each core returns a
partial (T, C) output = O_heads @ wo[rows of its heads]; host sums the 4 partials
per batch (the "all-reduce after c_proj").

v3 schedule: chunk-projection (C), attention-span (S) and output-projection (P)
phases are interleaved C0 S0 C1 P0 S1 C2 P1 S2 C3 P2 S3 P3 so the PE stream
never drains. Scalar runs only Sqrt/Exp/Copy (few activation-table loads);
reciprocals run on the DVE; elementwise casts/adds off the critical path run on
the otherwise-idle GpSimd. Latency tails (RMS apply broadcasts, last-head
softmax normalization) are deferred into the next phase's independent PE stream.
Diagonal attention blocks are restricted to their valid causal q-range.
"""
import sys
import math

sys.path.insert(0, "/opt/trn_rl_repo")

import numpy as np
import ml_dtypes
import concourse.bacc as bacc
import concourse.mybir as mybir
import concourse.tile as tile
from concourse.bass_utils import run_bass_kernel_spmd

P = 128
T = 2048
C = 2048
KO = C // P          # 16 contraction tiles
D = 128              # head dim
NQ = 4               # q heads per core
NK = 2               # kv heads per core
NF = NQ + NK         # 6 rope/rms feature blocks (4 q + 2 k)
FQ = NQ * D          # 512
FK = NK * D          # 256
TCH = 512            # chunk / span size
NCHUNK = T // TCH    # 4
SPAN = 512
KB = T // P          # 16 key blocks
SCALE = 1.0 / math.sqrt(D)
DEPTH = 3            # score-ahead software pipeline depth in attention

f32 = mybir.dt.float32
bf16 = mybir.dt.bfloat16

AF = mybir.ActivationFunctionType


def build():
    nc = bacc.Bacc("TRN2", target_bir_lowering=False)
    xT = nc.dram_tensor("xT", (C, T), bf16, kind="ExternalInput")
    wq = nc.dram_tensor("wq", (C, FQ), bf16, kind="ExternalInput")
    wk = nc.dram_tensor("wk", (C, FK), bf16, kind="ExternalInput")
    wv = nc.dram_tensor("wv", (C, FK), bf16, kind="ExternalInput")
    wo = nc.dram_tensor("wo", (FQ, C), bf16, kind="ExternalInput")
    cc = nc.dram_tensor("cc", (P, T), bf16, kind="ExternalInput")    # [cos; cos]
    ss = nc.dram_tensor("ss", (P, T), bf16, kind="ExternalInput")    # [sin; -sin]
    mask = nc.dram_tensor("mask", (P, P), bf16, kind="ExternalInput")  # [k, qq] = qq>=k
    y = nc.dram_tensor("y", (T, C), bf16, kind="ExternalOutput")

    xT_r = xT.rearrange("(ko p) t -> p ko t", p=P)
    wq_r = wq.rearrange("(ko p) f -> p ko f", p=P)
    wk_r = wk.rearrange("(ko p) f -> p ko f", p=P)
    wv_r = wv.rearrange("(ko p) f -> p ko f", p=P)
    wo_r = wo.rearrange("(ko p) n -> p ko n", p=P)

    with tile.TileContext(nc) as tc:
        with (
            tc.tile_pool(name="persist", bufs=1) as persist,
            tc.tile_pool(name="otp", bufs=2) as otp,
            tc.tile_pool(name="xp", bufs=2) as xp,
            tc.tile_pool(name="tpf", bufs=2) as tpf,
            tc.tile_pool(name="tps", bufs=2) as tps,
            tc.tile_pool(name="sqp", bufs=6) as sqp,
            tc.tile_pool(name="rstdp", bufs=6) as rstdp,
            tc.tile_pool(name="tpt", bufs=6) as tpt,
            tc.tile_pool(name="tpy", bufs=10) as tpy,
            tc.tile_pool(name="ps_mm", bufs=4, space="PSUM") as ps_mm,
            tc.tile_pool(name="ps_ot", bufs=2, space="PSUM") as ps_ot,
            tc.tile_pool(name="ps_sum", bufs=2, space="PSUM") as ps_sum,
        ):
            qk_rt = persist.tile([P, NF, T], bf16, tag="qk_rt")   # roped+normed qT/kT
            v_sb = persist.tile([P, KB, FK], bf16, tag="v_sb")    # V natural [t-part, kb, feat]
            cc_sb = persist.tile([P, T], bf16, tag="cc_sb")
            ss_sb = persist.tile([P, T], bf16, tag="ss_sb")
            mask_sb = persist.tile([P, P], bf16, tag="mask_sb")
            ones_col = persist.tile([P, 1], bf16, tag="ones_col")    # sums lhsT
            ones_row = persist.tile([1, P], bf16, tag="ones_row")    # bcast lhsT
            ones_f32 = persist.tile([P, 1], f32, tag="ones_f32")
            ones_row_f32 = persist.tile([1, P], f32, tag="ones_row_f32")
            wq_sb = persist.tile([P, KO, FQ], bf16, tag="wq_sb")
            wk_sb = persist.tile([P, KO, FK], bf16, tag="wk_sb")
            wv_sb = persist.tile([P, KO, FK], bf16, tag="wv_sb")
            wo_sb = persist.tile([P, NQ, C], bf16, tag="wo_sb")

            # split weight DMAs so the first matmuls wait only on their slice
            for fb in range(NQ):
                nc.sync.dma_start(wq_sb[:, :, fb * D : (fb + 1) * D],
                                  wq_r[:, :, fb * D : (fb + 1) * D])
            for fb in range(NK):
                nc.sync.dma_start(wk_sb[:, :, fb * D : (fb + 1) * D],
                                  wk_r[:, :, fb * D : (fb + 1) * D])
            nc.sync.dma_start(wv_sb[:], wv_r)
            nc.sync.dma_start(wo_sb[:], wo_r)
            nc.sync.dma_start(cc_sb[:], cc[:, :])
            nc.sync.dma_start(ss_sb[:], ss[:, :])
            nc.sync.dma_start(mask_sb[:], mask[:, :])
            nc.vector.memset(ones_f32[:], 1.0)
            nc.vector.memset(ones_row_f32[:], 1.0)
            nc.vector.tensor_copy(ones_col[:], ones_f32[:])
            nc.vector.tensor_copy(ones_row[:], ones_row_f32[:])

            def prefetch_x(c):
                t0 = c * TCH
                xt = xp.tile([P, KO, TCH], bf16, tag="xt")
                for ko in range(KO):
                    nc.sync.dma_start(xt[:, ko, :], xT_r[:, ko, t0 : t0 + TCH])
                return xt

            def emit_chunk(c, xt, norm_filler=None):
                """Project chunk c -> roped/normalized qT/kT + natural V.
                Returns thunks: deferred RMS-applies for q heads 1..3 (must run
                before span c's head h reads qk_rt[h])."""
                t0 = c * TCH
                segs = [None] * NF

                def emit_fb(fb):
                    if fb < NQ:
                        w_ap = wq_sb[:, :, fb * D : (fb + 1) * D]
                    else:
                        w_ap = wk_sb[:, :, (fb - NQ) * D : (fb - NQ + 1) * D]
                    pqk = ps_mm.tile([P, TCH], f32, tag="ps_mm")
                    for ko in range(KO):
                        nc.tensor.matmul(
                            pqk[:], w_ap[:, ko], xt[:, ko, :],
                            start=(ko == 0), stop=(ko == KO - 1),
                        )
                    # rope: raw copy on Scalar (table-safe), half-swap via DMA
                    raw = tpf.tile([P, TCH], f32, tag="raw")
                    nc.scalar.activation(raw[:], pqk[:], AF.Copy)
                    swp = tpf.tile([P, TCH], f32, tag="swp")
                    nc.sync.dma_start(swp[0:64, :], raw[64:128, :])
                    nc.sync.dma_start(swp[64:128, :], raw[0:64, :])
                    tmpa = tpf.tile([P, TCH], f32, tag="tmpa")
                    tmpb = tpf.tile([P, TCH], f32, tag="tmpb")
                    seg = qk_rt[:, fb, t0 : t0 + TCH]
                    nc.vector.tensor_mul(tmpa[:], pqk[:], cc_sb[:, t0 : t0 + TCH])
                    nc.vector.tensor_mul(tmpb[:], swp[:], ss_sb[:, t0 : t0 + TCH])
                    nc.gpsimd.tensor_add(seg, tmpa[:], tmpb[:])
                    sq = sqp.tile([P, TCH], bf16, tag="sq")
                    nc.vector.tensor_mul(sq[:], seg, seg)
                    segs[fb] = (seg, sq)

                # K features first so their rstd chains finish earliest
                for fb in (4, 5, 0, 1, 2, 3):
                    emit_fb(fb)
                    if fb == 4 and norm_filler is not None:
                        norm_filler()

                rstds = {}

                def emit_stat(fb):
                    pms = ps_sum.tile([1, TCH], f32, tag="ps_sum")
                    nc.tensor.matmul(pms[:], ones_col[:], segs[fb][1][:], start=True, stop=True)
                    # rstd = 1/sqrt(ms) = sqrt(D / pms); eps is negligible vs ms
                    inv = tps.tile([1, TCH], f32, tag="inv")
                    nc.vector.reciprocal_approx_fast(inv[:], pms[:])
                    rstd = rstdp.tile([1, TCH], bf16, tag="rstd")
                    nc.scalar.activation(rstd[:], inv[:], AF.Sqrt, scale=float(D))
                    rstds[fb] = rstd

                def emit_apply(fb):
                    pb = ps_mm.tile([P, TCH], f32, tag="ps_mm")
                    nc.tensor.matmul(pb[:], ones_row[:], rstds[fb][:], start=True, stop=True)
                    seg = segs[fb][0]
                    nc.vector.tensor_mul(seg, seg, pb[:])

                def emit_v(tb):
                    pv = ps_mm.tile([P, TCH], f32, tag="ps_mm")
                    for ko in range(KO):
                        nc.tensor.matmul(
                            pv[:, :FK],
                            xt[:, ko, tb * P : (tb + 1) * P],
                            wv_sb[:, ko, :],
                            start=(ko == 0), stop=(ko == KO - 1),
                        )
                    nc.vector.tensor_copy(
                        v_sb[:, c * (TCH // P) + tb, :], pv[:, :FK]
                    )

                # stats for K heads first (span c's scores need K normalized),
                # V matmuls as PE filler over the rstd latency chains
                emit_v(0)
                emit_stat(4)
                emit_v(1)
                emit_stat(5)
                emit_v(2)
                emit_stat(0)
                emit_apply(4)
                emit_v(3)
                emit_stat(1)
                emit_apply(5)
                emit_stat(2)
                emit_apply(0)
                emit_stat(3)
                deferred = [lambda fb=fb: emit_apply(fb) for fb in (1, 2, 3)]
                return deferred

            def emit_span(s, fillers):
                """Attention for q-span s. `fillers` are independent PE thunks
                sprinkled into the score stream (popped front-first). Returns
                the deferred normalization thunk of the last head."""
                q0 = s * SPAN
                nkb = 4 * s + 4
                ot_t = otp.tile([P, NQ, SPAN], bf16, tag="ot_t")
                pending = []

                def emit_norm(h, ot_ps, rec_r):
                    bc = ps_mm.tile([P, SPAN], f32, tag="ps_mm")
                    nc.tensor.matmul(bc[:], ones_row[:], rec_r[:], start=True, stop=True)
                    bc_sb = tps.tile([P, SPAN], f32, tag="bc_sb")
                    nc.vector.tensor_copy(bc_sb[:], bc[:])
                    nc.vector.tensor_mul(ot_t[:, h, :], ot_ps[:], bc_sb[:])

                for h in range(NQ):
                    j = h // 2
                    ot_ps = ps_ot.tile([P, SPAN], f32, tag="ot_ps")
                    sum_ps = ps_sum.tile([1, SPAN], f32, tag="ps_sum")
                    queue = []

                    def flush_one():
                        kb, off, vq, pt = queue.pop(0)
                        nc.tensor.matmul(
                            ot_ps[:, off:],
                            v_sb[:, kb, j * D : (j + 1) * D],
                            pt[:, :vq],
                            start=(kb == 0), stop=(kb == nkb - 1),
                            skip_group_check=True,
                        )
                        nc.tensor.matmul(
                            sum_ps[:, off:],
                            ones_col[:],
                            pt[:, :vq],
                            start=(kb == 0), stop=(kb == nkb - 1),
                            skip_group_check=True,
                        )

                    for kb in range(nkb):
                        r = kb - 4 * s           # >=0: diagonal block
                        off = P * r if r > 0 else 0
                        vq = SPAN - off
                        st = ps_mm.tile([P, SPAN], f32, tag="ps_mm")
                        nc.tensor.matmul(
                            st[:, :vq],
                            qk_rt[:, NQ + j, kb * P : (kb + 1) * P],
                            qk_rt[:, h, q0 + off : q0 + SPAN],
                            start=True, stop=True,
                        )
                        pt = tpt.tile([P, SPAN], bf16, tag="pt")
                        nc.scalar.activation(pt[:, :vq], st[:, :vq], AF.Exp, scale=SCALE)
                        if r >= 0:
                            nc.vector.tensor_mul(pt[:, :P], pt[:, :P], mask_sb[:])
                        queue.append((kb, off, vq, pt))
                        if fillers:
                            fillers.pop(0)()
                        if len(queue) > DEPTH:
                            flush_one()
                        if kb == DEPTH - 1 and pending:
                            emit_norm(*pending.pop())
                    while queue:
                        flush_one()
                    # DVE part of softmax normalization; the PE broadcast is
                    # deferred into the next head's (or phase's) PE stream
                    rec = tps.tile([1, SPAN], f32, tag="rec")
                    nc.vector.reciprocal_approx_fast(rec[:], sum_ps[:])
                    rec_r = tps.tile([1, SPAN], bf16, tag="rec_r")
                    nc.vector.tensor_copy(rec_r[:], rec[:])
                    pending.append((h, ot_ps, rec_r))
                last = pending.pop()
                return ot_t, (lambda: emit_norm(*last))

            def proj_thunks(c, ot_t):
                """Output projection for span c as independent PE thunks."""
                def one(tb, nch):
                    yps = ps_mm.tile([P, 512], f32, tag="ps_mm")
                    for h in range(NQ):
                        nc.tensor.matmul(
                            yps[:],
                            ot_t[:, h, tb * P : (tb + 1) * P],
                            wo_sb[:, h, nch * 512 : (nch + 1) * 512],
                            start=(h == 0), stop=(h == NQ - 1),
                        )
                    ysb = tpy.tile([P, 512], bf16, tag="ysb")
                    nc.vector.tensor_copy(ysb[:], yps[:])
                    nc.sync.dma_start(
                        y[(4 * c + tb) * P : (4 * c + tb + 1) * P,
                          nch * 512 : (nch + 1) * 512],
                        ysb[:],
                    )
                return [lambda tb=tb, nch=nch: one(tb, nch)
                        for tb in range(4) for nch in range(C // 512)]

            # C0 S0 C1 S1(+P0) C2 S2(+P1) C3 S3(+P2) P3: the output
            # projections ride as fillers inside the next span's score
            # stream, soaking up exp-latency bubbles.
            x0 = prefetch_x(0)
            d0 = emit_chunk(0, x0)
            x1 = prefetch_x(1)
            ot0, n0 = emit_span(0, d0)
            d1 = emit_chunk(1, x1, norm_filler=n0)
            x2 = prefetch_x(2)
            ot1, n1 = emit_span(1, d1 + proj_thunks(0, ot0))
            d2 = emit_chunk(2, x2, norm_filler=n1)
            x3 = prefetch_x(3)
            ot2, n2 = emit_span(2, d2 + proj_thunks(1, ot1))
            d3 = emit_chunk(3, x3, norm_filler=n2)
            ot3, n3 = emit_span(3, d3 + proj_thunks(2, ot2))
            n3()
            for t in proj_thunks(3, ot3):
                t()
    nc.compile()
    return nc


_NC_CACHE = None


def _get_nc():
    global _NC_CACHE
    if _NC_CACHE is None:
        _NC_CACHE = build()
    return _NC_CACHE


def _host_inputs(x, cos, sin, wq, wk, wv, wo):
    """Build the 8 per-core input maps."""
    bft = ml_dtypes.bfloat16
    cosT = np.ascontiguousarray(cos[0, :, 0, :].T).astype(np.float32)  # (64, T)
    sinT = np.ascontiguousarray(sin[0, :, 0, :].T).astype(np.float32)
    cc = np.concatenate([cosT, cosT], axis=0).astype(bft)  # (128, T)
    ss = np.concatenate([sinT, -sinT], axis=0).astype(bft)
    # mask[k, qq] = 1 if qq >= k (within the 128-wide diagonal sub-block)
    qq = np.arange(P)[None, :]
    kk = np.arange(P)[:, None]
    mask = (qq >= kk).astype(bft)  # (128, 128)

    xTs = [np.ascontiguousarray(x[b].T).astype(bft) for b in range(2)]
    wq16 = wq.astype(bft)
    wk16 = wk.astype(bft)
    wv16 = wv.astype(bft)
    wo16 = wo.astype(bft)
    in_maps = []
    for c in range(8):
        b, tp = divmod(c, 4)
        in_maps.append(
            {
                "xT": xTs[b],
                "wq": np.ascontiguousarray(wq16[:, tp * FQ : (tp + 1) * FQ]),
                "wk": np.ascontiguousarray(wk16[:, tp * FK : (tp + 1) * FK]),
                "wv": np.ascontiguousarray(wv16[:, tp * FK : (tp + 1) * FK]),
                "wo": np.ascontiguousarray(wo16[tp * FQ : (tp + 1) * FQ, :]),
                "cc": cc,
                "ss": ss,
                "mask": mask,
            }
        )
    return in_maps


def kernel(x, cos, sin, wq, wk, wv, wo, trace=False):
    x = np.asarray(x, dtype=np.float32)
    cos = np.asarray(cos, dtype=np.float32)
    sin = np.asarray(sin, dtype=np.float32)
    wq = np.asarray(wq, dtype=np.float32)
    wk = np.asarray(wk, dtype=np.float32)
    wv = np.asarray(wv, dtype=np.float32)
    wo = np.asarray(wo, dtype=np.float32)

    nc = _get_nc()
    in_maps = _host_inputs(x, cos, sin, wq, wk, wv, wo)
    res = run_bass_kernel_spmd(nc, in_maps, core_ids=list(range(8)), trace=trace)
    out = np.zeros((2, T, C), dtype=np.float32)
    for c in range(8):
        b = c // 4
        out[b] += res.results[c]["y"].astype(np.float32)
    if trace:
        return out, res
    return out



# revision 6
# speedup vs baseline: 1.0257x; 1.0257x over previous
"""Causal self-attention (RoPE + QK-RMSNorm, GQA 16q/8kv) Trainium2 Bass kernel.

Sharding: 8 cores = 2 batch x 4 tensor-parallel. Core c handles batch b=c//4 and
q-heads [4*tp, 4*tp+4), kv-heads [2*tp, 2*tp+2) where tp=c%4. Each core returns a
partial (T, C) output = O_heads @ wo[rows of its heads]; host sums the 4 partials
per batch (the "all-reduce after c_proj").

v4 schedule: all 4 projection chunks run first (dense back-to-back matmuls keep
the PE HAM-warm), then the 4 attention spans. Output projections of span s ride
as PE fillers inside span s+1's score stream. Within a span, the two q-heads
sharing a kv-head advance in interleaved waves (score -> exp -> lagged AV+sum)
so exp latency hides under the other head's matmuls. RMS stat/apply tails of
chunk c defer into chunk c+1's stream. Inputs are pre-tiled host-side into
DMA-contiguous layouts; transfers are issued on both HWDGE queues (sync+scalar),
first-needed first.
"""
import sys
import math

sys.path.insert(0, "/opt/trn_rl_repo")

import numpy as np
import ml_dtypes
import concourse.bacc as bacc
import concourse.mybir as mybir
import concourse.tile as tile
from concourse.bass_utils import run_bass_kernel_spmd

P = 128
T = 2048
C = 2048
KO = C // P          # 16 contraction tiles
D = 128              # head dim
NQ = 4               # q heads per core
NK = 2               # kv heads per core
NF = NQ + NK         # 6 rope/rms feature blocks (4 q + 2 k)
FQ = NQ * D          # 512
FK = NK * D          # 256
TCH = 512            # chunk / span size
NCHUNK = T // TCH    # 4
SPAN = 512
KB = T // P          # 16 key blocks
SCALE = 1.0 / math.sqrt(D)
LAG = 2              # AV/sum lag (in waves) behind the score stream

f32 = mybir.dt.float32
bf16 = mybir.dt.bfloat16

AF = mybir.ActivationFunctionType


def build():
    nc = bacc.Bacc("TRN2", target_bir_lowering=False)
    # pre-tiled DMA-contiguous input layouts (see _host_inputs)
    xTt = nc.dram_tensor("xTt", (NCHUNK, KO, P, TCH), bf16, kind="ExternalInput")
    wqt = nc.dram_tensor("wqt", (NQ, P, KO, D), bf16, kind="ExternalInput")
    wkt = nc.dram_tensor("wkt", (NK, 4, P, KO // 4, D), bf16, kind="ExternalInput")
    wvt = nc.dram_tensor("wvt", (4, P, KO // 4, FK), bf16, kind="ExternalInput")
    wot = nc.dram_tensor("wot", (NQ, P, C), bf16, kind="ExternalInput")
    cc = nc.dram_tensor("cc", (P, T), bf16, kind="ExternalInput")    # [cos; cos]
    ss = nc.dram_tensor("ss", (P, T), bf16, kind="ExternalInput")    # [sin; -sin]
    mask = nc.dram_tensor("mask", (P, P), bf16, kind="ExternalInput")  # [k, qq] = qq>=k
    y = nc.dram_tensor("y", (T, C), bf16, kind="ExternalOutput")

    with tile.TileContext(nc) as tc:
        with (
            tc.tile_pool(name="persist", bufs=1) as persist,
            tc.tile_pool(name="otp", bufs=2) as otp,
            tc.tile_pool(name="xp", bufs=2) as xp,
            tc.tile_pool(name="tpf", bufs=3) as tpf,
            tc.tile_pool(name="tps", bufs=3) as tps,
            tc.tile_pool(name="sqp", bufs=7) as sqp,
            tc.tile_pool(name="rstdp", bufs=8) as rstdp,
            tc.tile_pool(name="tpt", bufs=8) as tpt,
            tc.tile_pool(name="tpy", bufs=6) as tpy,
            tc.tile_pool(name="ps_mm", bufs=4, space="PSUM") as ps_mm,
            tc.tile_pool(name="ps_ot", bufs=2, space="PSUM") as ps_ot,
            tc.tile_pool(name="ps_sum", bufs=2, space="PSUM") as ps_sum,
        ):
            qk_rt = persist.tile([P, NF, T], bf16, tag="qk_rt")   # roped+normed qT/kT
            v_sb = persist.tile([P, KB, FK], bf16, tag="v_sb")    # V natural [t-part, kb, feat]
            cc_sb = persist.tile([P, T], bf16, tag="cc_sb")
            ss_sb = persist.tile([P, T], bf16, tag="ss_sb")
            mask_sb = persist.tile([P, P], bf16, tag="mask_sb")
            ones_col = persist.tile([P, 1], bf16, tag="ones_col")    # sums lhsT
            ones_row = persist.tile([1, P], bf16, tag="ones_row")    # bcast lhsT
            ones_f32 = persist.tile([P, 1], f32, tag="ones_f32")
            ones_row_f32 = persist.tile([1, P], f32, tag="ones_row_f32")
            wq_sb = persist.tile([P, NQ, KO, D], bf16, tag="wq_sb")
            wk_sb = persist.tile([P, NK, KO, D], bf16, tag="wk_sb")
            wv_sb = persist.tile([P, KO, FK], bf16, tag="wv_sb")
            wo_sb = persist.tile([P, NQ, C], bf16, tag="wo_sb")

            xts = [None] * NCHUNK

            def issue_x(c, eng_pair):
                xt = xp.tile([P, KO, TCH], bf16, tag="xt")
                for ko in range(KO):
                    eng = eng_pair[ko % len(eng_pair)]
                    eng.dma_start(xt[:, ko, :], xTt[c, ko])
                xts[c] = xt

            # -- startup DMA schedule: first-needed first, split across both
            #    HWDGE issue queues (sync + scalar). wk in ko-quarters so the
            #    first matmuls wait only on small transfers.
            for g in range(4):
                eng = nc.sync if g % 2 == 0 else nc.scalar
                eng.dma_start(wk_sb[:, 0, 4 * g : 4 * (g + 1), :], wkt[0, g])
            issue_x(0, (nc.sync, nc.scalar))
            for g in range(4):
                eng = nc.scalar if g % 2 == 0 else nc.sync
                eng.dma_start(wk_sb[:, 1, 4 * g : 4 * (g + 1), :], wkt[1, g])
            nc.sync.dma_start(cc_sb[:, : T // 2], cc[:, : T // 2])
            nc.scalar.dma_start(ss_sb[:, : T // 2], ss[:, : T // 2])
            nc.sync.dma_start(cc_sb[:, T // 2 :], cc[:, T // 2 :])
            nc.scalar.dma_start(ss_sb[:, T // 2 :], ss[:, T // 2 :])
            for fb in range(NQ):
                eng = nc.sync if fb % 2 == 0 else nc.scalar
                eng.dma_start(wq_sb[:, fb], wqt[fb])
            for g in range(4):
                eng = nc.scalar if g % 2 == 0 else nc.sync
                eng.dma_start(wv_sb[:, 4 * g : 4 * (g + 1), :], wvt[g])
            nc.sync.dma_start(mask_sb[:], mask[:, :])
            nc.vector.memset(ones_f32[:], 1.0)
            nc.vector.memset(ones_row_f32[:], 1.0)
            nc.vector.tensor_copy(ones_col[:], ones_f32[:])
            nc.vector.tensor_copy(ones_row[:], ones_row_f32[:])

            # ---------------- chunk projection ----------------
            def emit_chunk(c, deferred_in):
                """Project chunk c. `deferred_in`: stat/apply thunks from the
                previous chunk, placed into this chunk's early PE stream.
                Returns this chunk's deferred thunks."""
                t0 = c * TCH
                xt = xts[c]
                segs = [None] * NF
                rstds = {}
                dq = list(deferred_in)

                def pop_deferred():
                    if dq:
                        dq.pop(0)()

                def emit_fb(fb):
                    if fb < NQ:
                        w_ap = wq_sb[:, fb]
                    else:
                        w_ap = wk_sb[:, fb - NQ]
                    pqk = ps_mm.tile([P, TCH], f32, tag="ps_mm")
                    for ko in range(KO):
                        nc.tensor.matmul(
                            pqk[:], w_ap[:, ko], xt[:, ko, :],
                            start=(ko == 0), stop=(ko == KO - 1),
                        )
                    # rope: raw copy on Scalar, half-swap via DMA (one half
                    # issued per HWDGE engine so both run concurrently)
                    raw = tpf.tile([P, TCH], f32, tag="raw")
                    nc.scalar.activation(raw[:], pqk[:], AF.Copy)
                    swp = tpf.tile([P, TCH], f32, tag="swp")
                    nc.sync.dma_start(swp[0:64, :], raw[64:128, :])
                    nc.scalar.dma_start(swp[64:128, :], raw[0:64, :])
                    tmpa = tpf.tile([P, TCH], f32, tag="tmpa")
                    tmpb = tpf.tile([P, TCH], f32, tag="tmpb")
                    seg = qk_rt[:, fb, t0 : t0 + TCH]
                    nc.vector.tensor_mul(tmpa[:], pqk[:], cc_sb[:, t0 : t0 + TCH])
                    nc.vector.tensor_mul(tmpb[:], swp[:], ss_sb[:, t0 : t0 + TCH])
                    nc.gpsimd.tensor_add(seg, tmpa[:], tmpb[:])
                    sq = sqp.tile([P, TCH], bf16, tag="sq")
                    nc.vector.tensor_mul(sq[:], seg, seg)
                    segs[fb] = (seg, sq)

                def emit_stat(fb):
                    pms = ps_sum.tile([1, TCH], f32, tag="ps_sum")
                    nc.tensor.matmul(pms[:], ones_col[:], segs[fb][1][:], start=True, stop=True)
                    # rstd = 1/sqrt(ms) = sqrt(D / pms); eps negligible vs ms
                    inv = tps.tile([1, TCH], f32, tag="inv")
                    nc.vector.reciprocal_approx_fast(inv[:], pms[:])
                    rstd = rstdp.tile([1, TCH], bf16, tag="rstd")
                    nc.scalar.activation(rstd[:], inv[:], AF.Sqrt, scale=float(D))
                    rstds[fb] = rstd

                def emit_apply(fb):
                    pb = ps_mm.tile([P, TCH], f32, tag="ps_mm")
                    nc.tensor.matmul(pb[:], ones_row[:], rstds[fb][:], start=True, stop=True)
                    seg = segs[fb][0]
                    nc.vector.tensor_mul(seg, seg, pb[:])

                def emit_v(tb):
                    pv = ps_mm.tile([P, TCH], f32, tag="ps_mm")
                    for ko in range(KO):
                        nc.tensor.matmul(
                            pv[:, :FK],
                            xt[:, ko, tb * P : (tb + 1) * P],
                            wv_sb[:, ko, :],
                            start=(ko == 0), stop=(ko == KO - 1),
                        )
                    nc.vector.tensor_copy(
                        v_sb[:, c * (TCH // P) + tb, :], pv[:, :FK]
                    )

                # dense PE stream; stat/apply chains placed late enough that
                # their sq inputs (swap-DMA latency ~9us) are ready
                emit_fb(4)
                pop_deferred()                      # prev stat2
                emit_fb(5)
                pop_deferred()                      # prev apply2
                emit_fb(0)
                pop_deferred()                      # prev stat3
                emit_v(0)
                pop_deferred()                      # prev apply3
                emit_fb(1)
                emit_v(1)
                emit_stat(4)
                emit_stat(5)
                emit_fb(2)
                emit_stat(0)
                emit_v(2)
                emit_apply(4)
                emit_apply(5)
                emit_fb(3)
                emit_stat(1)
                emit_v(3)
                emit_apply(0)
                emit_apply(1)
                deferred = [
                    lambda: emit_stat(2),
                    lambda: emit_apply(2),
                    lambda: emit_stat(3),
                    lambda: emit_apply(3),
                ]
                return deferred

            # ---------------- attention span ----------------
            def emit_norm(ot_t, h, ot_ps, rec_r):
                bc = ps_mm.tile([P, SPAN], f32, tag="ps_mm")
                nc.tensor.matmul(bc[:], ones_row[:], rec_r[:], start=True, stop=True)
                bc_sb = tps.tile([P, SPAN], f32, tag="bc_sb")
                nc.vector.tensor_copy(bc_sb[:], bc[:])
                nc.vector.tensor_mul(ot_t[:, h, :], ot_ps[:], bc_sb[:])

            def emit_span(s, fillers):
                """Attention for q-span s, two q-heads per kv-head advancing in
                interleaved waves. `fillers`: independent PE thunks popped one
                per wave. Returns (ot_t, deferred norm thunks for pair 1)."""
                q0 = s * SPAN
                nkb = 4 * s + 4
                ot_t = otp.tile([P, NQ, SPAN], bf16, tag="ot_t")
                pending = []

                for j in range(NK):  # kv head = head pair
                    # free previous pair's ot banks before this pair's AVs
                    while pending:
                        emit_norm(ot_t, *pending.pop(0))
                    hs = (2 * j, 2 * j + 1)
                    ot_ps = {h: ps_ot.tile([P, SPAN], f32, tag="ot_ps", name="ot_ps")
                             for h in hs}
                    sum_ps = {h: ps_sum.tile([1, SPAN], f32, tag="ps_sum", name="sum_ps")
                              for h in hs}
                    queue = []

                    def flush_one():
                        h, kb, off, vq, pt = queue.pop(0)
                        nc.tensor.matmul(
                            ot_ps[h][:, off:],
                            v_sb[:, kb, j * D : (j + 1) * D],
                            pt[:, :vq],
                            start=(kb == 0), stop=(kb == nkb - 1),
                            skip_group_check=True,
                        )
                        nc.tensor.matmul(
                            sum_ps[h][:, off:],
                            ones_col[:],
                            pt[:, :vq],
                            start=(kb == 0), stop=(kb == nkb - 1),
                            skip_group_check=True,
                        )

                    for kb in range(nkb):
                        r = kb - 4 * s           # >=0: diagonal block group
                        off = P * r if r > 0 else 0
                        vq = SPAN - off
                        for h in hs:
                            st = ps_mm.tile([P, SPAN], f32, tag="ps_mm")
                            nc.tensor.matmul(
                                st[:, :vq],
                                qk_rt[:, NQ + j, kb * P : (kb + 1) * P],
                                qk_rt[:, h, q0 + off : q0 + SPAN],
                                start=True, stop=True,
                            )
                            pt = tpt.tile([P, SPAN], bf16, tag="pt")
                            nc.scalar.activation(pt[:, :vq], st[:, :vq], AF.Exp, scale=SCALE)
                            if r >= 0:
                                nc.vector.tensor_mul(pt[:, :P], pt[:, :P], mask_sb[:])
                            queue.append((h, kb, off, vq, pt))
                        while queue and queue[0][1] <= kb - LAG:
                            flush_one()
                        if fillers:
                            fillers.pop(0)()
                    while queue:
                        flush_one()
                    # softmax denominators -> reciprocal on DVE; the PE
                    # broadcast is deferred (next pair / next span's stream)
                    for h in hs:
                        rec = tps.tile([1, SPAN], f32, tag="rec")
                        nc.vector.reciprocal_approx_fast(rec[:], sum_ps[h][:])
                        rec_r = tps.tile([1, SPAN], bf16, tag="rec_r")
                        nc.vector.tensor_copy(rec_r[:], rec[:])
                        pending.append((h, ot_ps[h], rec_r))
                # drain any fillers that didn't fit in the wave slots
                while fillers:
                    fillers.pop(0)()
                return ot_t, pending

            def proj_thunks(c, ot_t):
                """Output projection for span c as independent PE thunks."""
                def one(tb, nch):
                    yps = ps_mm.tile([P, 512], f32, tag="ps_mm")
                    for h in range(NQ):
                        nc.tensor.matmul(
                            yps[:],
                            ot_t[:, h, tb * P : (tb + 1) * P],
                            wo_sb[:, h, nch * 512 : (nch + 1) * 512],
                            start=(h == 0), stop=(h == NQ - 1),
                        )
                    ysb = tpy.tile([P, 512], bf16, tag="ysb")
                    nc.vector.tensor_copy(ysb[:], yps[:])
                    nc.sync.dma_start(
                        y[(4 * c + tb) * P : (4 * c + tb + 1) * P,
                          nch * 512 : (nch + 1) * 512],
                        ysb[:],
                    )
                return [lambda tb=tb, nch=nch: one(tb, nch)
                        for tb in range(4) for nch in range(C // 512)]

            # ---------------- program ----------------
            # chunks first (dense PE, HAM-warm), prefetching x one ahead
            issue_x(1, (nc.sync,))
            d = emit_chunk(0, [])
            issue_x(2, (nc.sync,))
            d = emit_chunk(1, d)
            issue_x(3, (nc.sync,))
            # wo needed first inside span 1's filler stream
            for h in range(NQ):
                eng = nc.sync if h % 2 == 0 else nc.scalar
                eng.dma_start(wo_sb[:, h], wot[h])
            d = emit_chunk(2, d)
            d = emit_chunk(3, d)

            # spans; span s-1's output projection rides in span s's stream.
            # chunk3's stat3/apply3 tail needs ~6us of swap-DMA latency past
            # chunk3's end -- pad so it pops a few waves into span 0.
            noop = lambda: None
            d = d[:2] + [noop] * 3 + d[2:]
            ot0, n0 = emit_span(0, d)
            f1 = [lambda n=n: emit_norm(ot0, *n) for n in n0] + proj_thunks(0, ot0)
            ot1, n1 = emit_span(1, f1)
            f2 = [lambda n=n: emit_norm(ot1, *n) for n in n1] + proj_thunks(1, ot1)
            ot2, n2 = emit_span(2, f2)
            f3 = [lambda n=n: emit_norm(ot2, *n) for n in n2] + proj_thunks(2, ot2)
            ot3, n3 = emit_span(3, f3)
            for n in n3:
                emit_norm(ot3, *n)
            for t in proj_thunks(3, ot3):
                t()
    nc.compile()
    return nc


_NC_CACHE = None


def _get_nc():
    global _NC_CACHE
    if _NC_CACHE is None:
        _NC_CACHE = build()
    return _NC_CACHE


def _host_inputs(x, cos, sin, wq, wk, wv, wo):
    """Build the 8 per-core input maps with DMA-contiguous pre-tiled layouts."""
    bft = ml_dtypes.bfloat16
    cosT = np.ascontiguousarray(cos[0, :, 0, :].T).astype(np.float32)  # (64, T)
    sinT = np.ascontiguousarray(sin[0, :, 0, :].T).astype(np.float32)
    cc = np.ascontiguousarray(np.concatenate([cosT, cosT], axis=0)).astype(bft)  # (128, T)
    ss = np.ascontiguousarray(np.concatenate([sinT, -sinT], axis=0)).astype(bft)
    # mask[k, qq] = 1 if qq >= k (within the 128-wide diagonal sub-block)
    qq = np.arange(P)[None, :]
    kk = np.arange(P)[:, None]
    mask = np.ascontiguousarray((qq >= kk).astype(bft))  # (128, 128)

    # xTt[c, ko, p, t] = x[b][c*TCH+t, ko*P+p]
    xTts = []
    for b in range(2):
        xb = x[b].astype(bft)                       # (T, C)
        a = xb.reshape(NCHUNK, TCH, KO, P)          # [c, t, ko, p]
        xTts.append(np.ascontiguousarray(a.transpose(0, 2, 3, 1)))  # [c, ko, p, t]

    wq16 = wq.astype(bft)
    wk16 = wk.astype(bft)
    wv16 = wv.astype(bft)
    wo16 = wo.astype(bft)
    in_maps = []
    for core in range(8):
        b, tp = divmod(core, 4)
        wq_s = wq16[:, tp * FQ : (tp + 1) * FQ]     # (C, FQ)
        wk_s = wk16[:, tp * FK : (tp + 1) * FK]     # (C, FK)
        wv_s = wv16[:, tp * FK : (tp + 1) * FK]
        wo_s = wo16[tp * FQ : (tp + 1) * FQ, :]     # (FQ, C)
        # wqt[fb, p, ko, d] = wq_s[ko*P+p, fb*D+d]
        a = wq_s.reshape(KO, P, NQ, D)
        wqt = np.ascontiguousarray(a.transpose(2, 1, 0, 3))          # (NQ, P, KO, D)
        # wkt[kh, g, p, kog, d] = wk_s[(4g+kog)*P+p, kh*D+d]
        a = wk_s.reshape(4, KO // 4, P, NK, D)                       # [g, kog, p, kh, d]
        wkt = np.ascontiguousarray(a.transpose(3, 0, 2, 1, 4))       # (NK, 4, P, KO//4, D)
        # wvt[g, p, kog, f] = wv_s[(4g+kog)*P+p, f]
        a = wv_s.reshape(4, KO // 4, P, FK)
        wvt = np.ascontiguousarray(a.transpose(0, 2, 1, 3))          # (4, P, KO//4, FK)
        # wot[h, p, n] = wo_s[h*D+p, n]
        wot = np.ascontiguousarray(wo_s.reshape(NQ, P, C))
        in_maps.append(
            {
                "xTt": xTts[b],
                "wqt": wqt,
                "wkt": wkt,
                "wvt": wvt,
                "wot": wot,
                "cc": cc,
                "ss": ss,
                "mask": mask,
            }
        )
    return in_maps


def kernel(x, cos, sin, wq, wk, wv, wo, trace=False):
    x = np.asarray(x, dtype=np.float32)
    cos = np.asarray(cos, dtype=np.float32)
    sin = np.asarray(sin, dtype=np.float32)
    wq = np.asarray(wq, dtype=np.float32)
    wk = np.asarray(wk, dtype=np.float32)
    wv = np.asarray(wv, dtype=np.float32)
    wo = np.asarray(wo, dtype=np.float32)

    nc = _get_nc()
    in_maps = _host_inputs(x, cos, sin, wq, wk, wv, wo)
    res = run_bass_kernel_spmd(nc, in_maps, core_ids=list(range(8)), trace=trace)
    out = np.zeros((2, T, C), dtype=np.float32)
    for c in range(8):
        b = c // 4
        out[b] += res.results[c]["y"].astype(np.float32)
    if trace:
        return out, res
    return out


# revision 11
# speedup vs baseline: 1.0969x; 1.0694x over previous
"""Causal self-attention (RoPE + QK-RMSNorm, GQA 16q/8kv) Trainium2 Bass kernel.

Sharding: 8 cores = 2 batch x 4 tensor-parallel. Core c handles batch b=c//4 and
q-heads [4*tp, 4*tp+4), kv-heads [2*tp, 2*tp+2) where tp=c%4. Each core returns a
partial (T, C) output = O_heads @ wo[rows of its heads]; host sums the 4 partials
per batch (the "all-reduce after c_proj").

v4 schedule: all 4 projection chunks run first (dense back-to-back matmuls keep
the PE HAM-warm), then the 4 attention spans. Output projections of span s ride
as PE fillers inside span s+1's score stream. Within a span, the two q-heads
sharing a kv-head advance in interleaved waves (score -> exp -> lagged AV+sum)
so exp latency hides under the other head's matmuls. RMS stat/apply tails of
chunk c defer into chunk c+1's stream. Inputs are pre-tiled host-side into
DMA-contiguous layouts; transfers are issued on both HWDGE queues (sync+scalar),
first-needed first.
"""
import sys
import math

sys.path.insert(0, "/opt/trn_rl_repo")

import numpy as np
import ml_dtypes
import concourse.bacc as bacc
import concourse.mybir as mybir
import concourse.tile as tile
from concourse.bass_utils import run_bass_kernel_spmd

P = 128
T = 2048
C = 2048
KO = C // P          # 16 contraction tiles
D = 128              # head dim
NQ = 4               # q heads per core
NK = 2               # kv heads per core
NF = NQ + NK         # 6 rope/rms feature blocks (4 q + 2 k)
FQ = NQ * D          # 512
FK = NK * D          # 256
TCH = 512            # chunk / span size
NCHUNK = T // TCH    # 4
SPAN = 512
KB = T // P          # 16 key blocks
SCALE = 1.0 / math.sqrt(D)
LAG = 2              # AV/sum lag (in waves) behind the score stream

f32 = mybir.dt.float32
bf16 = mybir.dt.bfloat16

AF = mybir.ActivationFunctionType


def build():
    nc = bacc.Bacc("TRN2", target_bir_lowering=False)
    # pre-tiled DMA-contiguous input layouts (see _host_inputs)
    xTt = nc.dram_tensor("xTt", (NCHUNK, KO, P, TCH), bf16, kind="ExternalInput")
    wqt = nc.dram_tensor("wqt", (NQ, P, KO, D), bf16, kind="ExternalInput")
    wkt = nc.dram_tensor("wkt", (NK, 4, P, KO // 4, D), bf16, kind="ExternalInput")
    wvt = nc.dram_tensor("wvt", (4, P, KO // 4, FK), bf16, kind="ExternalInput")
    wot = nc.dram_tensor("wot", (NQ, P, C), bf16, kind="ExternalInput")
    cc = nc.dram_tensor("cc", (P, T), bf16, kind="ExternalInput")    # [cos; cos]
    ss = nc.dram_tensor("ss", (P, T), bf16, kind="ExternalInput")    # [sin; -sin]
    mask = nc.dram_tensor("mask", (P, P), bf16, kind="ExternalInput")  # [k, qq] = qq>=k
    y = nc.dram_tensor("y", (T, C), bf16, kind="ExternalOutput")

    with tile.TileContext(nc) as tc:
        with (
            tc.tile_pool(name="persist", bufs=1) as persist,
            tc.tile_pool(name="otp", bufs=2) as otp,
            tc.tile_pool(name="xp", bufs=2) as xp,
            tc.tile_pool(name="tpf", bufs=3) as tpf,
            tc.tile_pool(name="tps", bufs=3) as tps,
            tc.tile_pool(name="sqp", bufs=7) as sqp,
            tc.tile_pool(name="rstdp", bufs=8) as rstdp,
            tc.tile_pool(name="tpt", bufs=8) as tpt,
            tc.tile_pool(name="tpy", bufs=6) as tpy,
            tc.tile_pool(name="ps_mm", bufs=4, space="PSUM") as ps_mm,
            tc.tile_pool(name="ps_ot", bufs=2, space="PSUM") as ps_ot,
            tc.tile_pool(name="ps_sum", bufs=2, space="PSUM") as ps_sum,
        ):
            qk_rt = persist.tile([P, NF, T], bf16, tag="qk_rt")   # roped+normed qT/kT
            v_sb = persist.tile([P, KB, FK], bf16, tag="v_sb")    # V natural [t-part, kb, feat]
            cc_sb = persist.tile([P, T], bf16, tag="cc_sb")
            ss_sb = persist.tile([P, T], bf16, tag="ss_sb")
            mask_sb = persist.tile([P, P], bf16, tag="mask_sb")
            ones_col = persist.tile([P, 1], bf16, tag="ones_col")    # sums lhsT
            ones_row = persist.tile([1, P], bf16, tag="ones_row")    # bcast lhsT
            ones_f32 = persist.tile([P, 1], f32, tag="ones_f32")
            ones_row_f32 = persist.tile([1, P], f32, tag="ones_row_f32")
            wq_sb = persist.tile([P, NQ, KO, D], bf16, tag="wq_sb")
            wk_sb = persist.tile([P, NK, KO, D], bf16, tag="wk_sb")
            wv_sb = persist.tile([P, KO, FK], bf16, tag="wv_sb")
            wo_sb = persist.tile([P, NQ, C], bf16, tag="wo_sb")

            xts = [None] * NCHUNK

            def issue_x(c, eng_pair):
                xt = xp.tile([P, KO, TCH], bf16, tag="xt")
                for ko in range(KO):
                    eng = eng_pair[ko % len(eng_pair)]
                    eng.dma_start(xt[:, ko, :], xTt[c, ko])
                xts[c] = xt

            # -- startup DMA schedule: first-needed first, split across both
            #    HWDGE issue queues (sync + scalar). wk in ko-quarters so the
            #    first matmuls wait only on small transfers.
            for g in range(4):
                eng = nc.sync if g % 2 == 0 else nc.scalar
                eng.dma_start(wk_sb[:, 0, 4 * g : 4 * (g + 1), :], wkt[0, g])
            issue_x(0, (nc.sync, nc.scalar))
            for g in range(4):
                eng = nc.scalar if g % 2 == 0 else nc.sync
                eng.dma_start(wk_sb[:, 1, 4 * g : 4 * (g + 1), :], wkt[1, g])
            nc.sync.dma_start(cc_sb[:, : T // 2], cc[:, : T // 2])
            nc.scalar.dma_start(ss_sb[:, : T // 2], ss[:, : T // 2])
            nc.sync.dma_start(cc_sb[:, T // 2 :], cc[:, T // 2 :])
            nc.scalar.dma_start(ss_sb[:, T // 2 :], ss[:, T // 2 :])
            for fb in range(NQ):
                eng = nc.sync if fb % 2 == 0 else nc.scalar
                eng.dma_start(wq_sb[:, fb], wqt[fb])
            for g in range(4):
                eng = nc.scalar if g % 2 == 0 else nc.sync
                eng.dma_start(wv_sb[:, 4 * g : 4 * (g + 1), :], wvt[g])
            nc.sync.dma_start(mask_sb[:], mask[:, :])
            nc.vector.memset(ones_f32[:], 1.0)
            nc.vector.memset(ones_row_f32[:], 1.0)
            nc.vector.tensor_copy(ones_col[:], ones_f32[:])
            nc.vector.tensor_copy(ones_row[:], ones_row_f32[:])

            # ---------------- chunk projection ----------------
            def emit_chunk(c, deferred_in):
                """Project chunk c. `deferred_in`: stat/apply thunks from the
                previous chunk, placed into this chunk's early PE stream.
                Returns this chunk's deferred thunks."""
                t0 = c * TCH
                xt = xts[c]
                segs = [None] * NF
                rstds = {}
                dq = list(deferred_in)

                def pop_deferred():
                    if dq:
                        dq.pop(0)()

                def emit_fb(fb):
                    if fb < NQ:
                        w_ap = wq_sb[:, fb]
                    else:
                        w_ap = wk_sb[:, fb - NQ]
                    pqk = ps_mm.tile([P, TCH], f32, tag="ps_mm")
                    for ko in range(KO):
                        nc.tensor.matmul(
                            pqk[:], w_ap[:, ko], xt[:, ko, :],
                            start=(ko == 0), stop=(ko == KO - 1),
                        )
                    # rope: raw copy on Scalar, half-swap via DMA (one half
                    # issued per HWDGE engine so both run concurrently)
                    raw = tpf.tile([P, TCH], f32, tag="raw")
                    nc.scalar.activation(raw[:], pqk[:], AF.Copy)
                    # quarter swaps across both HWDGE queues: shorter transfer
                    # latency keeps the sq->stat chains off the PE critical path
                    swp = tpf.tile([P, TCH], f32, tag="swp")
                    nc.sync.dma_start(swp[0:32, :], raw[64:96, :])
                    nc.scalar.dma_start(swp[32:64, :], raw[96:128, :])
                    nc.sync.dma_start(swp[64:96, :], raw[0:32, :])
                    nc.scalar.dma_start(swp[96:128, :], raw[32:64, :])
                    tmpa = tpf.tile([P, TCH], f32, tag="tmpa")
                    tmpb = tpf.tile([P, TCH], f32, tag="tmpb")
                    seg = qk_rt[:, fb, t0 : t0 + TCH]
                    nc.vector.tensor_mul(tmpa[:], pqk[:], cc_sb[:, t0 : t0 + TCH])
                    nc.vector.tensor_mul(tmpb[:], swp[:], ss_sb[:, t0 : t0 + TCH])
                    nc.gpsimd.tensor_add(seg, tmpa[:], tmpb[:])
                    sq = sqp.tile([P, TCH], bf16, tag="sq")
                    nc.vector.tensor_mul(sq[:], seg, seg)
                    segs[fb] = (seg, sq)

                def emit_stat(fb):
                    pms = ps_sum.tile([1, TCH], f32, tag="ps_sum")
                    nc.tensor.matmul(pms[:], ones_col[:], segs[fb][1][:], start=True, stop=True)
                    # rstd = 1/sqrt(ms) = sqrt(D / pms); eps negligible vs ms
                    inv = tps.tile([1, TCH], f32, tag="inv")
                    nc.vector.reciprocal_approx_fast(inv[:], pms[:])
                    rstd = rstdp.tile([1, TCH], bf16, tag="rstd")
                    nc.scalar.activation(rstd[:], inv[:], AF.Sqrt, scale=float(D))
                    rstds[fb] = rstd

                def emit_apply(fb):
                    pb = ps_mm.tile([P, TCH], f32, tag="ps_mm")
                    nc.tensor.matmul(pb[:], ones_row[:], rstds[fb][:], start=True, stop=True)
                    seg = segs[fb][0]
                    nc.vector.tensor_mul(seg, seg, pb[:])

                def emit_v(tb):
                    pv = ps_mm.tile([P, TCH], f32, tag="ps_mm")
                    for ko in range(KO):
                        nc.tensor.matmul(
                            pv[:, :FK],
                            xt[:, ko, tb * P : (tb + 1) * P],
                            wv_sb[:, ko, :],
                            start=(ko == 0), stop=(ko == KO - 1),
                        )
                    nc.vector.tensor_copy(
                        v_sb[:, c * (TCH // P) + tb, :], pv[:, :FK]
                    )

                # dense fb block first (max slack for the swap-DMA chains),
                # then V blocks with the stat/apply chains interleaved
                for fb in (4, 5, 0, 1, 2, 3):
                    emit_fb(fb)
                pop_deferred()                      # prev apply2
                pop_deferred()                      # prev apply3
                # prefetch next x now: these sync-queue issues sit behind this
                # chunk's swap issues and drain during the V blocks
                if c + 1 < NCHUNK:
                    issue_x(c + 1, (nc.sync,))
                if c == 1:
                    for h in range(NQ):
                        eng = nc.sync if h % 2 == 0 else nc.scalar
                        eng.dma_start(wo_sb[:, h], wot[h])
                emit_v(0)
                emit_stat(4)
                emit_v(1)
                emit_stat(5)
                emit_stat(0)
                emit_v(2)
                emit_stat(1)
                emit_apply(4)
                emit_apply(5)
                emit_v(3)
                emit_stat(2)
                emit_apply(0)
                emit_stat(3)
                emit_apply(1)
                # only the (table-free) applies defer into the next phase, so
                # no Sqrt table load ever lands inside the span exp stream
                deferred = [
                    lambda: emit_apply(2),
                    lambda: emit_apply(3),
                ]
                return deferred

            # ---------------- attention span ----------------
            def emit_norm(ot_t, h, ot_ps, rec_r):
                bc = ps_mm.tile([P, SPAN], f32, tag="ps_mm")
                nc.tensor.matmul(bc[:], ones_row[:], rec_r[:], start=True, stop=True)
                bc_sb = tps.tile([P, SPAN], f32, tag="bc_sb")
                nc.vector.tensor_copy(bc_sb[:], bc[:])
                nc.vector.tensor_mul(ot_t[:, h, :], ot_ps[:], bc_sb[:])

            def emit_span(s, fillers):
                """Attention for q-span s, two q-heads per kv-head advancing in
                interleaved waves. `fillers`: independent PE thunks popped one
                per wave. Returns (ot_t, deferred norm thunks for pair 1)."""
                q0 = s * SPAN
                nkb = 4 * s + 4
                ot_t = otp.tile([P, NQ, SPAN], bf16, tag="ot_t")
                pending = []

                for j in range(NK):  # kv head = head pair
                    # free previous pair's ot banks before this pair's AVs;
                    # a filler ahead of each norm covers the DVE rec latency
                    while pending:
                        if fillers:
                            fillers.pop(0)()
                        emit_norm(ot_t, *pending.pop(0))
                    hs = (2 * j, 2 * j + 1)
                    ot_ps = {h: ps_ot.tile([P, SPAN], f32, tag="ot_ps", name="ot_ps")
                             for h in hs}
                    sum_ps = {h: ps_sum.tile([1, SPAN], f32, tag="ps_sum", name="sum_ps")
                              for h in hs}
                    queue = []

                    def flush_one():
                        h, kb, off, vq, pt = queue.pop(0)
                        nc.tensor.matmul(
                            ot_ps[h][:, off:],
                            v_sb[:, kb, j * D : (j + 1) * D],
                            pt[:, :vq],
                            start=(kb == 0), stop=(kb == nkb - 1),
                            skip_group_check=True,
                        )
                        nc.tensor.matmul(
                            sum_ps[h][:, off:],
                            ones_col[:],
                            pt[:, :vq],
                            start=(kb == 0), stop=(kb == nkb - 1),
                            skip_group_check=True,
                        )

                    for kb in range(nkb):
                        r = kb - 4 * s           # >=0: diagonal block group
                        off = P * r if r > 0 else 0
                        vq = SPAN - off
                        for h in hs:
                            st = ps_mm.tile([P, SPAN], f32, tag="ps_mm")
                            nc.tensor.matmul(
                                st[:, :vq],
                                qk_rt[:, NQ + j, kb * P : (kb + 1) * P],
                                qk_rt[:, h, q0 + off : q0 + SPAN],
                                start=True, stop=True,
                            )
                            pt = tpt.tile([P, SPAN], bf16, tag="pt")
                            nc.scalar.activation(pt[:, :vq], st[:, :vq], AF.Exp, scale=SCALE)
                            if r >= 0:
                                nc.vector.tensor_mul(pt[:, :P], pt[:, :P], mask_sb[:])
                            queue.append((h, kb, off, vq, pt))
                        while queue and queue[0][1] <= kb - LAG:
                            flush_one()
                        if fillers:
                            fillers.pop(0)()
                    while queue:
                        flush_one()
                    # softmax denominators -> reciprocal on DVE; the PE
                    # broadcast is deferred (next pair / next span's stream)
                    for h in hs:
                        rec = tps.tile([1, SPAN], f32, tag="rec")
                        nc.vector.reciprocal_approx_fast(rec[:], sum_ps[h][:])
                        rec_r = tps.tile([1, SPAN], bf16, tag="rec_r")
                        nc.vector.tensor_copy(rec_r[:], rec[:])
                        pending.append((h, ot_ps[h], rec_r))
                # drain any fillers that didn't fit in the wave slots
                while fillers:
                    fillers.pop(0)()
                return ot_t, pending

            def proj_thunks(c, ot_t, split_dma=False):
                """Output projection for span c as independent PE thunks."""
                def one(tb, nch):
                    yps = ps_mm.tile([P, 512], f32, tag="ps_mm")
                    for h in range(NQ):
                        nc.tensor.matmul(
                            yps[:],
                            ot_t[:, h, tb * P : (tb + 1) * P],
                            wo_sb[:, h, nch * 512 : (nch + 1) * 512],
                            start=(h == 0), stop=(h == NQ - 1),
                        )
                    ysb = tpy.tile([P, 512], bf16, tag="ysb")
                    nc.vector.tensor_copy(ysb[:], yps[:])
                    rows = slice((4 * c + tb) * P, (4 * c + tb + 1) * P)
                    if split_dma:
                        # halve the final transfers so the kernel tail isn't
                        # gated by one long DMA
                        nc.sync.dma_start(
                            y[rows, nch * 512 : nch * 512 + 256], ysb[:, :256])
                        nc.scalar.dma_start(
                            y[rows, nch * 512 + 256 : (nch + 1) * 512], ysb[:, 256:])
                    else:
                        nc.sync.dma_start(
                            y[rows, nch * 512 : (nch + 1) * 512], ysb[:])
                return [lambda tb=tb, nch=nch: one(tb, nch)
                        for tb in range(4) for nch in range(C // 512)]

            # ---------------- program ----------------
            # chunks first (dense PE, HAM-warm); x prefetch happens inside
            # each chunk, after its swap-DMA issues
            d = emit_chunk(0, [])
            d = emit_chunk(1, d)
            d = emit_chunk(2, d)
            d = emit_chunk(3, d)

            # spans; span s-1's output projection rides in span s's stream.
            # chunk3's deferred applies pad a wave into span 0 so their rstd
            # chains are ready.
            noop = lambda: None
            d = [noop] + d[:1] + [noop] + d[1:]
            ot0, n0 = emit_span(0, d)
            f1 = [lambda n=n: emit_norm(ot0, *n) for n in n0] + proj_thunks(0, ot0)
            ot1, n1 = emit_span(1, f1)
            f2 = [lambda n=n: emit_norm(ot1, *n) for n in n1] + proj_thunks(1, ot1)
            ot2, n2 = emit_span(2, f2)
            f3 = [lambda n=n: emit_norm(ot2, *n) for n in n2] + proj_thunks(2, ot2)
            ot3, n3 = emit_span(3, f3)
            for n in n3:
                emit_norm(ot3, *n)
            for t in proj_thunks(3, ot3, split_dma=True):
                t()
    nc.compile()
    return nc


_NC_CACHE = None


def _get_nc():
    global _NC_CACHE
    if _NC_CACHE is None:
        _NC_CACHE = build()
    return _NC_CACHE


def _host_inputs(x, cos, sin, wq, wk, wv, wo):
    """Build the 8 per-core input maps with DMA-contiguous pre-tiled layouts."""
    bft = ml_dtypes.bfloat16
    cosT = np.ascontiguousarray(cos[0, :, 0, :].T).astype(np.float32)  # (64, T)
    sinT = np.ascontiguousarray(sin[0, :, 0, :].T).astype(np.float32)
    cc = np.ascontiguousarray(np.concatenate([cosT, cosT], axis=0)).astype(bft)  # (128, T)
    ss = np.ascontiguousarray(np.concatenate([sinT, -sinT], axis=0)).astype(bft)
    # mask[k, qq] = 1 if qq >= k (within the 128-wide diagonal sub-block)
    qq = np.arange(P)[None, :]
    kk = np.arange(P)[:, None]
    mask = np.ascontiguousarray((qq >= kk).astype(bft))  # (128, 128)

    # xTt[c, ko, p, t] = x[b][c*TCH+t, ko*P+p]
    xTts = []
    for b in range(2):
        xb = x[b].astype(bft)                       # (T, C)
        a = xb.reshape(NCHUNK, TCH, KO, P)          # [c, t, ko, p]
        xTts.append(np.ascontiguousarray(a.transpose(0, 2, 3, 1)))  # [c, ko, p, t]

    wq16 = wq.astype(bft)
    wk16 = wk.astype(bft)
    wv16 = wv.astype(bft)
    wo16 = wo.astype(bft)
    in_maps = []
    for core in range(8):
        b, tp = divmod(core, 4)
        wq_s = wq16[:, tp * FQ : (tp + 1) * FQ]     # (C, FQ)
        wk_s = wk16[:, tp * FK : (tp + 1) * FK]     # (C, FK)
        wv_s = wv16[:, tp * FK : (tp + 1) * FK]
        wo_s = wo16[tp * FQ : (tp + 1) * FQ, :]     # (FQ, C)
        # wqt[fb, p, ko, d] = wq_s[ko*P+p, fb*D+d]
        a = wq_s.reshape(KO, P, NQ, D)
        wqt = np.ascontiguousarray(a.transpose(2, 1, 0, 3))          # (NQ, P, KO, D)
        # wkt[kh, g, p, kog, d] = wk_s[(4g+kog)*P+p, kh*D+d]
        a = wk_s.reshape(4, KO // 4, P, NK, D)                       # [g, kog, p, kh, d]
        wkt = np.ascontiguousarray(a.transpose(3, 0, 2, 1, 4))       # (NK, 4, P, KO//4, D)
        # wvt[g, p, kog, f] = wv_s[(4g+kog)*P+p, f]
        a = wv_s.reshape(4, KO // 4, P, FK)
        wvt = np.ascontiguousarray(a.transpose(0, 2, 1, 3))          # (4, P, KO//4, FK)
        # wot[h, p, n] = wo_s[h*D+p, n]
        wot = np.ascontiguousarray(wo_s.reshape(NQ, P, C))
        in_maps.append(
            {
                "xTt": xTts[b],
                "wqt": wqt,
                "wkt": wkt,
                "wvt": wvt,
                "wot": wot,
                "cc": cc,
                "ss": ss,
                "mask": mask,
            }
        )
    return in_maps


def kernel(x, cos, sin, wq, wk, wv, wo, trace=False):
    x = np.asarray(x, dtype=np.float32)
    cos = np.asarray(cos, dtype=np.float32)
    sin = np.asarray(sin, dtype=np.float32)
    wq = np.asarray(wq, dtype=np.float32)
    wk = np.asarray(wk, dtype=np.float32)
    wv = np.asarray(wv, dtype=np.float32)
    wo = np.asarray(wo, dtype=np.float32)

    nc = _get_nc()
    in_maps = _host_inputs(x, cos, sin, wq, wk, wv, wo)
    res = run_bass_kernel_spmd(nc, in_maps, core_ids=list(range(8)), trace=trace)
    out = np.zeros((2, T, C), dtype=np.float32)
    for c in range(8):
        b = c // 4
        out[b] += res.results[c]["y"].astype(np.float32)
    if trace:
        return out, res
    return out


# revision 14
# speedup vs baseline: 1.1028x; 1.0053x over previous
"""Causal self-attention (RoPE + QK-RMSNorm, GQA 16q/8kv) Trainium2 Bass kernel.

Sharding: 8 cores = 2 batch x 4 tensor-parallel. Core c handles batch b=c//4 and
q-heads [4*tp, 4*tp+4), kv-heads [2*tp, 2*tp+2) where tp=c%4. Each core returns a
partial (T, C) output = O_heads @ wo[rows of its heads]; host sums the 4 partials
per batch (the "all-reduce after c_proj").

v4 schedule: all 4 projection chunks run first (dense back-to-back matmuls keep
the PE HAM-warm), then the 4 attention spans. Output projections of span s ride
as PE fillers inside span s+1's score stream. Within a span, the two q-heads
sharing a kv-head advance in interleaved waves (score -> exp -> lagged AV+sum)
so exp latency hides under the other head's matmuls. RMS stat/apply tails of
chunk c defer into chunk c+1's stream. Inputs are pre-tiled host-side into
DMA-contiguous layouts; transfers are issued on both HWDGE queues (sync+scalar),
first-needed first.
"""
import sys
import math

sys.path.insert(0, "/opt/trn_rl_repo")

import numpy as np
import ml_dtypes
import concourse.bacc as bacc
import concourse.mybir as mybir
import concourse.tile as tile
from concourse.bass_utils import run_bass_kernel_spmd

P = 128
T = 2048
C = 2048
KO = C // P          # 16 contraction tiles
D = 128              # head dim
NQ = 4               # q heads per core
NK = 2               # kv heads per core
NF = NQ + NK         # 6 rope/rms feature blocks (4 q + 2 k)
FQ = NQ * D          # 512
FK = NK * D          # 256
TCH = 512            # chunk / span size
NCHUNK = T // TCH    # 4
SPAN = 512
KB = T // P          # 16 key blocks
SCALE = 1.0 / math.sqrt(D)
LAG = 2              # AV/sum lag (in waves) behind the score stream

f32 = mybir.dt.float32
bf16 = mybir.dt.bfloat16

AF = mybir.ActivationFunctionType


def build():
    nc = bacc.Bacc("TRN2", target_bir_lowering=False)
    # pre-tiled DMA-contiguous input layouts (see _host_inputs)
    xTt = nc.dram_tensor("xTt", (NCHUNK, KO, P, TCH), bf16, kind="ExternalInput")
    wqt = nc.dram_tensor("wqt", (NQ, P, KO, D), bf16, kind="ExternalInput")
    wkt = nc.dram_tensor("wkt", (NK, 4, P, KO // 4, D), bf16, kind="ExternalInput")
    wvt = nc.dram_tensor("wvt", (4, P, KO // 4, FK), bf16, kind="ExternalInput")
    wot = nc.dram_tensor("wot", (NQ, P, C), bf16, kind="ExternalInput")
    cc = nc.dram_tensor("cc", (P, T), bf16, kind="ExternalInput")    # [cos; cos]
    ss = nc.dram_tensor("ss", (P, T), bf16, kind="ExternalInput")    # [sin; -sin]
    mask = nc.dram_tensor("mask", (P, P), bf16, kind="ExternalInput")  # [k, qq] = qq>=k
    y = nc.dram_tensor("y", (T, C), bf16, kind="ExternalOutput")

    with tile.TileContext(nc) as tc:
        with (
            tc.tile_pool(name="persist", bufs=1) as persist,
            tc.tile_pool(name="otp", bufs=2) as otp,
            tc.tile_pool(name="xp", bufs=2) as xp,
            tc.tile_pool(name="tpf", bufs=3) as tpf,
            tc.tile_pool(name="tpw", bufs=6) as tpw,
            tc.tile_pool(name="tps", bufs=3) as tps,
            tc.tile_pool(name="sqp", bufs=7) as sqp,
            tc.tile_pool(name="rstdp", bufs=8) as rstdp,
            tc.tile_pool(name="tpt", bufs=8) as tpt,
            tc.tile_pool(name="tpy", bufs=6) as tpy,
            tc.tile_pool(name="ps_mm", bufs=4, space="PSUM") as ps_mm,
            tc.tile_pool(name="ps_ot", bufs=2, space="PSUM") as ps_ot,
            tc.tile_pool(name="ps_sum", bufs=2, space="PSUM") as ps_sum,
        ):
            qk_rt = persist.tile([P, NF, T], bf16, tag="qk_rt")   # roped+normed qT/kT
            v_sb = persist.tile([P, KB, FK], bf16, tag="v_sb")    # V natural [t-part, kb, feat]
            cc_sb = persist.tile([P, T], bf16, tag="cc_sb")
            ss_sb = persist.tile([P, T], bf16, tag="ss_sb")
            mask_sb = persist.tile([P, P], bf16, tag="mask_sb")
            ones_col = persist.tile([P, 1], bf16, tag="ones_col")    # sums lhsT
            ones_row = persist.tile([1, P], bf16, tag="ones_row")    # bcast lhsT
            ones_f32 = persist.tile([P, 1], f32, tag="ones_f32")
            ones_row_f32 = persist.tile([1, P], f32, tag="ones_row_f32")
            wq_sb = persist.tile([P, NQ, KO, D], bf16, tag="wq_sb")
            wk_sb = persist.tile([P, NK, KO, D], bf16, tag="wk_sb")
            wv_sb = persist.tile([P, KO, FK], bf16, tag="wv_sb")
            wo_sb = persist.tile([P, NQ, C], bf16, tag="wo_sb")

            xts = [None] * NCHUNK

            def issue_x(c, eng_pair):
                xt = xp.tile([P, KO, TCH], bf16, tag="xt")
                for ko in range(KO):
                    eng = eng_pair[ko % len(eng_pair)]
                    eng.dma_start(xt[:, ko, :], xTt[c, ko])
                xts[c] = xt

            # -- startup DMA schedule: first-needed first, split across both
            #    HWDGE issue queues (sync + scalar). wk in ko-quarters so the
            #    first matmuls wait only on small transfers.
            for g in range(4):
                eng = nc.sync if g % 2 == 0 else nc.scalar
                eng.dma_start(wk_sb[:, 0, 4 * g : 4 * (g + 1), :], wkt[0, g])
            issue_x(0, (nc.sync, nc.scalar))
            for g in range(4):
                eng = nc.scalar if g % 2 == 0 else nc.sync
                eng.dma_start(wk_sb[:, 1, 4 * g : 4 * (g + 1), :], wkt[1, g])
            nc.sync.dma_start(cc_sb[:, : T // 2], cc[:, : T // 2])
            nc.scalar.dma_start(ss_sb[:, : T // 2], ss[:, : T // 2])
            nc.sync.dma_start(cc_sb[:, T // 2 :], cc[:, T // 2 :])
            nc.scalar.dma_start(ss_sb[:, T // 2 :], ss[:, T // 2 :])
            for fb in range(NQ):
                eng = nc.sync if fb % 2 == 0 else nc.scalar
                eng.dma_start(wq_sb[:, fb], wqt[fb])
            for g in range(4):
                eng = nc.scalar if g % 2 == 0 else nc.sync
                eng.dma_start(wv_sb[:, 4 * g : 4 * (g + 1), :], wvt[g])
            nc.sync.dma_start(mask_sb[:], mask[:, :])
            nc.vector.memset(ones_f32[:], 1.0)
            nc.vector.memset(ones_row_f32[:], 1.0)
            nc.vector.tensor_copy(ones_col[:], ones_f32[:])
            nc.vector.tensor_copy(ones_row[:], ones_row_f32[:])

            # ---------------- chunk projection ----------------
            def emit_chunk(c, deferred_in):
                """Project chunk c. `deferred_in`: stat/apply thunks from the
                previous chunk, placed into this chunk's early PE stream.
                Returns this chunk's deferred thunks."""
                t0 = c * TCH
                xt = xts[c]
                segs = [None] * NF
                rstds = {}
                dq = list(deferred_in)

                def pop_deferred():
                    if dq:
                        dq.pop(0)()

                swps = {}
                tmpas = {}

                def emit_fb_a(fb):
                    """Projection matmuls + rope prologue: raw copy, swap-DMA
                    issue, cos product. The sin-side products batch in
                    emit_fb_b once all swaps are in flight, so neither the
                    DVE nor scalar queue ever head-blocks on DMA latency."""
                    if fb < NQ:
                        w_ap = wq_sb[:, fb]
                    else:
                        w_ap = wk_sb[:, fb - NQ]
                    pqk = ps_mm.tile([P, TCH], f32, tag="ps_mm")
                    for ko in range(KO):
                        nc.tensor.matmul(
                            pqk[:], w_ap[:, ko], xt[:, ko, :],
                            start=(ko == 0), stop=(ko == KO - 1),
                        )
                    raw = tpf.tile([P, TCH], bf16, tag="raw")
                    nc.scalar.activation(raw[:], pqk[:], AF.Copy)
                    # quarter swaps across both HWDGE queues (bf16: 32KB each)
                    swp = tpw.tile([P, TCH], bf16, tag="swp")
                    nc.sync.dma_start(swp[0:32, :], raw[64:96, :])
                    nc.scalar.dma_start(swp[32:64, :], raw[96:128, :])
                    nc.sync.dma_start(swp[64:96, :], raw[0:32, :])
                    nc.scalar.dma_start(swp[96:128, :], raw[32:64, :])
                    tmpa = tpw.tile([P, TCH], bf16, tag="tmpa")
                    nc.vector.tensor_mul(tmpa[:], pqk[:], cc_sb[:, t0 : t0 + TCH])
                    swps[fb] = swp
                    tmpas[fb] = tmpa

                def emit_fb_b(fb):
                    tmpb = tpf.tile([P, TCH], bf16, tag="tmpb")
                    nc.vector.tensor_mul(tmpb[:], swps[fb][:], ss_sb[:, t0 : t0 + TCH])
                    seg = qk_rt[:, fb, t0 : t0 + TCH]
                    nc.vector.tensor_add(seg, tmpas[fb][:], tmpb[:])
                    sq = sqp.tile([P, TCH], bf16, tag="sq")
                    nc.vector.tensor_mul(sq[:], seg, seg)
                    segs[fb] = (seg, sq)

                def emit_stat(fb):
                    pms = ps_sum.tile([1, TCH], f32, tag="ps_sum")
                    nc.tensor.matmul(pms[:], ones_col[:], segs[fb][1][:], start=True, stop=True)
                    # rstd = 1/sqrt(ms) = sqrt(D / pms); eps negligible vs ms
                    inv = tps.tile([1, TCH], f32, tag="inv")
                    nc.vector.reciprocal_approx_fast(inv[:], pms[:])
                    rstd = rstdp.tile([1, TCH], bf16, tag="rstd")
                    nc.scalar.activation(rstd[:], inv[:], AF.Sqrt, scale=float(D))
                    rstds[fb] = rstd

                def emit_apply(fb):
                    pb = ps_mm.tile([P, TCH], f32, tag="ps_mm")
                    nc.tensor.matmul(pb[:], ones_row[:], rstds[fb][:], start=True, stop=True)
                    seg = segs[fb][0]
                    nc.vector.tensor_mul(seg, seg, pb[:])

                def emit_v(tb):
                    pv = ps_mm.tile([P, TCH], f32, tag="ps_mm")
                    for ko in range(KO):
                        nc.tensor.matmul(
                            pv[:, :FK],
                            xt[:, ko, tb * P : (tb + 1) * P],
                            wv_sb[:, ko, :],
                            start=(ko == 0), stop=(ko == KO - 1),
                        )
                    nc.vector.tensor_copy(
                        v_sb[:, c * (TCH // P) + tb, :], pv[:, :FK]
                    )

                # dense fb block first (max slack for the swap-DMA chains),
                # then V blocks with the stat/apply chains interleaved
                for fb in (4, 5, 0, 1, 2, 3):
                    emit_fb_a(fb)
                pop_deferred()                      # prev apply2
                pop_deferred()                      # prev apply3
                for fb in (4, 5, 0, 1, 2, 3):
                    emit_fb_b(fb)
                # prefetch next x now: these sync-queue issues sit behind this
                # chunk's swap issues and drain during the V blocks
                if c + 1 < NCHUNK:
                    issue_x(c + 1, (nc.sync,))
                if c == 1:
                    for h in range(NQ):
                        eng = nc.sync if h % 2 == 0 else nc.scalar
                        eng.dma_start(wo_sb[:, h], wot[h])
                emit_v(0)
                emit_stat(4)
                emit_v(1)
                emit_stat(5)
                emit_stat(0)
                emit_v(2)
                emit_stat(1)
                emit_apply(4)
                emit_apply(5)
                emit_v(3)
                emit_stat(2)
                emit_apply(0)
                emit_stat(3)
                emit_apply(1)
                # only the (table-free) applies defer into the next phase, so
                # no Sqrt table load ever lands inside the span exp stream
                deferred = [
                    lambda: emit_apply(2),
                    lambda: emit_apply(3),
                ]
                return deferred

            # ---------------- attention span ----------------
            def emit_norm(ot_t, h, ot_ps, rec_r):
                bc = ps_mm.tile([P, SPAN], f32, tag="ps_mm")
                nc.tensor.matmul(bc[:], ones_row[:], rec_r[:], start=True, stop=True)
                bc_sb = tps.tile([P, SPAN], f32, tag="bc_sb")
                nc.vector.tensor_copy(bc_sb[:], bc[:])
                nc.vector.tensor_mul(ot_t[:, h, :], ot_ps[:], bc_sb[:])

            def emit_span(s, fillers):
                """Attention for q-span s, two q-heads per kv-head advancing in
                interleaved waves. `fillers`: independent PE thunks popped one
                per wave. Returns (ot_t, deferred norm thunks for pair 1)."""
                q0 = s * SPAN
                nkb = 4 * s + 4
                ot_t = otp.tile([P, NQ, SPAN], bf16, tag="ot_t")
                pending = []

                for j in range(NK):  # kv head = head pair
                    # free previous pair's ot banks before this pair's AVs;
                    # a filler ahead of each norm covers the DVE rec latency
                    while pending:
                        if fillers:
                            fillers.pop(0)()
                        emit_norm(ot_t, *pending.pop(0))
                    hs = (2 * j, 2 * j + 1)
                    ot_ps = {h: ps_ot.tile([P, SPAN], f32, tag="ot_ps", name="ot_ps")
                             for h in hs}
                    sum_ps = {h: ps_sum.tile([1, SPAN], f32, tag="ps_sum", name="sum_ps")
                              for h in hs}
                    queue = []

                    def flush_one():
                        h, kb, off, vq, pt = queue.pop(0)
                        nc.tensor.matmul(
                            ot_ps[h][:, off:],
                            v_sb[:, kb, j * D : (j + 1) * D],
                            pt[:, :vq],
                            start=(kb == 0), stop=(kb == nkb - 1),
                            skip_group_check=True,
                        )
                        nc.tensor.matmul(
                            sum_ps[h][:, off:],
                            ones_col[:],
                            pt[:, :vq],
                            start=(kb == 0), stop=(kb == nkb - 1),
                            skip_group_check=True,
                        )

                    for kb in range(nkb):
                        r = kb - 4 * s           # >=0: diagonal block group
                        off = P * r if r > 0 else 0
                        vq = SPAN - off
                        for h in hs:
                            st = ps_mm.tile([P, SPAN], f32, tag="ps_mm")
                            nc.tensor.matmul(
                                st[:, :vq],
                                qk_rt[:, NQ + j, kb * P : (kb + 1) * P],
                                qk_rt[:, h, q0 + off : q0 + SPAN],
                                start=True, stop=True,
                            )
                            pt = tpt.tile([P, SPAN], bf16, tag="pt")
                            nc.scalar.activation(pt[:, :vq], st[:, :vq], AF.Exp, scale=SCALE)
                            if r >= 0:
                                nc.vector.tensor_mul(pt[:, :P], pt[:, :P], mask_sb[:])
                            queue.append((h, kb, off, vq, pt))
                        while queue and queue[0][1] <= kb - LAG:
                            flush_one()
                        if fillers:
                            fillers.pop(0)()
                    while queue:
                        flush_one()
                    # softmax denominators -> reciprocal on DVE; the PE
                    # broadcast is deferred (next pair / next span's stream)
                    for h in hs:
                        rec = tps.tile([1, SPAN], f32, tag="rec")
                        nc.vector.reciprocal_approx_fast(rec[:], sum_ps[h][:])
                        rec_r = tps.tile([1, SPAN], bf16, tag="rec_r")
                        nc.vector.tensor_copy(rec_r[:], rec[:])
                        pending.append((h, ot_ps[h], rec_r))
                # drain any fillers that didn't fit in the wave slots
                while fillers:
                    fillers.pop(0)()
                return ot_t, pending

            def proj_thunks(c, ot_t, split_dma=False):
                """Output projection for span c as independent PE thunks."""
                def one(tb, nch):
                    yps = ps_mm.tile([P, 512], f32, tag="ps_mm")
                    for h in range(NQ):
                        nc.tensor.matmul(
                            yps[:],
                            ot_t[:, h, tb * P : (tb + 1) * P],
                            wo_sb[:, h, nch * 512 : (nch + 1) * 512],
                            start=(h == 0), stop=(h == NQ - 1),
                        )
                    ysb = tpy.tile([P, 512], bf16, tag="ysb")
                    nc.vector.tensor_copy(ysb[:], yps[:])
                    rows = slice((4 * c + tb) * P, (4 * c + tb + 1) * P)
                    if split_dma:
                        # halve the final transfers so the kernel tail isn't
                        # gated by one long DMA
                        nc.sync.dma_start(
                            y[rows, nch * 512 : nch * 512 + 256], ysb[:, :256])
                        nc.scalar.dma_start(
                            y[rows, nch * 512 + 256 : (nch + 1) * 512], ysb[:, 256:])
                    else:
                        nc.sync.dma_start(
                            y[rows, nch * 512 : (nch + 1) * 512], ysb[:])
                return [lambda tb=tb, nch=nch: one(tb, nch)
                        for tb in range(4) for nch in range(C // 512)]

            # ---------------- program ----------------
            # chunks first (dense PE, HAM-warm); x prefetch happens inside
            # each chunk, after its swap-DMA issues
            d = emit_chunk(0, [])
            d = emit_chunk(1, d)
            d = emit_chunk(2, d)
            d = emit_chunk(3, d)

            # spans; span s-1's output projection rides in span s's stream.
            # chunk3's deferred applies pad a wave into span 0 so their rstd
            # chains are ready.
            noop = lambda: None
            d = [noop] + d[:1] + [noop] + d[1:]
            ot0, n0 = emit_span(0, d)
            f1 = [lambda n=n: emit_norm(ot0, *n) for n in n0] + proj_thunks(0, ot0)
            ot1, n1 = emit_span(1, f1)
            f2 = [lambda n=n: emit_norm(ot1, *n) for n in n1] + proj_thunks(1, ot1)
            ot2, n2 = emit_span(2, f2)
            f3 = [lambda n=n: emit_norm(ot2, *n) for n in n2] + proj_thunks(2, ot2)
            ot3, n3 = emit_span(3, f3)
            for n in n3:
                emit_norm(ot3, *n)
            for t in proj_thunks(3, ot3, split_dma=True):
                t()
    nc.compile()
    return nc


_NC_CACHE = None


def _get_nc():
    global _NC_CACHE
    if _NC_CACHE is None:
        _NC_CACHE = build()
    return _NC_CACHE


def _host_inputs(x, cos, sin, wq, wk, wv, wo):
    """Build the 8 per-core input maps with DMA-contiguous pre-tiled layouts."""
    bft = ml_dtypes.bfloat16
    cosT = np.ascontiguousarray(cos[0, :, 0, :].T).astype(np.float32)  # (64, T)
    sinT = np.ascontiguousarray(sin[0, :, 0, :].T).astype(np.float32)
    cc = np.ascontiguousarray(np.concatenate([cosT, cosT], axis=0)).astype(bft)  # (128, T)
    ss = np.ascontiguousarray(np.concatenate([sinT, -sinT], axis=0)).astype(bft)
    # mask[k, qq] = 1 if qq >= k (within the 128-wide diagonal sub-block)
    qq = np.arange(P)[None, :]
    kk = np.arange(P)[:, None]
    mask = np.ascontiguousarray((qq >= kk).astype(bft))  # (128, 128)

    # xTt[c, ko, p, t] = x[b][c*TCH+t, ko*P+p]
    xTts = []
    for b in range(2):
        xb = x[b].astype(bft)                       # (T, C)
        a = xb.reshape(NCHUNK, TCH, KO, P)          # [c, t, ko, p]
        xTts.append(np.ascontiguousarray(a.transpose(0, 2, 3, 1)))  # [c, ko, p, t]

    wq16 = wq.astype(bft)
    wk16 = wk.astype(bft)
    wv16 = wv.astype(bft)
    wo16 = wo.astype(bft)
    in_maps = []
    for core in range(8):
        b, tp = divmod(core, 4)
        wq_s = wq16[:, tp * FQ : (tp + 1) * FQ]     # (C, FQ)
        wk_s = wk16[:, tp * FK : (tp + 1) * FK]     # (C, FK)
        wv_s = wv16[:, tp * FK : (tp + 1) * FK]
        wo_s = wo16[tp * FQ : (tp + 1) * FQ, :]     # (FQ, C)
        # wqt[fb, p, ko, d] = wq_s[ko*P+p, fb*D+d]
        a = wq_s.reshape(KO, P, NQ, D)
        wqt = np.ascontiguousarray(a.transpose(2, 1, 0, 3))          # (NQ, P, KO, D)
        # wkt[kh, g, p, kog, d] = wk_s[(4g+kog)*P+p, kh*D+d]
        a = wk_s.reshape(4, KO // 4, P, NK, D)                       # [g, kog, p, kh, d]
        wkt = np.ascontiguousarray(a.transpose(3, 0, 2, 1, 4))       # (NK, 4, P, KO//4, D)
        # wvt[g, p, kog, f] = wv_s[(4g+kog)*P+p, f]
        a = wv_s.reshape(4, KO // 4, P, FK)
        wvt = np.ascontiguousarray(a.transpose(0, 2, 1, 3))          # (4, P, KO//4, FK)
        # wot[h, p, n] = wo_s[h*D+p, n]
        wot = np.ascontiguousarray(wo_s.reshape(NQ, P, C))
        in_maps.append(
            {
                "xTt": xTts[b],
                "wqt": wqt,
                "wkt": wkt,
                "wvt": wvt,
                "wot": wot,
                "cc": cc,
                "ss": ss,
                "mask": mask,
            }
        )
    return in_maps


def kernel(x, cos, sin, wq, wk, wv, wo, trace=False):
    x = np.asarray(x, dtype=np.float32)
    cos = np.asarray(cos, dtype=np.float32)
    sin = np.asarray(sin, dtype=np.float32)
    wq = np.asarray(wq, dtype=np.float32)
    wk = np.asarray(wk, dtype=np.float32)
    wv = np.asarray(wv, dtype=np.float32)
    wo = np.asarray(wo, dtype=np.float32)

    nc = _get_nc()
    in_maps = _host_inputs(x, cos, sin, wq, wk, wv, wo)
    res = run_bass_kernel_spmd(nc, in_maps, core_ids=list(range(8)), trace=trace)
    out = np.zeros((2, T, C), dtype=np.float32)
    for c in range(8):
        b = c // 4
        out[b] += res.results[c]["y"].astype(np.float32)
    if trace:
        return out, res
    return out
